# revision 1
# baseline (speedup 1.0000x reference)
"""Trainium2 Bass kernel for nn_MessageAttentionPassing.

Math (reference):
    xp  = x.transpose(0,2,3,1)            # [B, N, T, CIN]
    h   = xp @ W1 + b1                    # [B, N, T, HID]
    mv  = h @ W2[:HID]                    # dest part
    mh  = h @ W2[HID:]                    # src part
    a    = attention[:, 0]                # [B, N(i), N(j), T]
    asum = a.sum(axis=2)                  # [B, N, T]
    upd = asum[...,None]*(mv+b2) + einsum('bijt,bjtc->bitc', a, mh)
    out = upd.transpose(0,3,1,2)          # [B, COUT, N, T]

Sharding: 8 cores = (batch b in {0,1}) x (dest-node quarter q in {0..3}).
Each core loads the full x[b] (to build h/mh over all source nodes j) and
its 32-row attention slice.  Inputs are node-rotated by i0 = 32*q on the
host so every core runs the IDENTICAL program with its own dest chunk at
rotated positions 0..31 (run_bass_kernel_spmd requires one shared BIR
program across cores).

On-chip layouts (per core, rotated node axis n, free dim (n,t) n-major):
    xT2  [128, 1536]  partition=(half,cin)  free=local (n,t), halves n<64 / n>=64
    hT2  [128, 1536]  partition=(half,hid)  same free split
    mh   [128, 1536]  partition=j (all 128) free=(t,c) t-major
    attT [128,  768]  partition=j           free=(i,t) i-major (DMA-transposed load)
    per-t einsum: psum[c, i] = mh_t[j,c].T-weights @ attT_t[j,i]
"""

import os
import sys
import numpy as np

if "/opt/trn_rl_repo" not in sys.path:
    sys.path.insert(0, "/opt/trn_rl_repo")

B, CIN, N, T, COUT, HID = 2, 64, 128, 24, 64, 64
NI = N // 4          # dest-node chunk per core: 32
NT = N * T           # 3072
F2 = NT // 2         # 1536
IT = NI * T          # 768

_PROGRAM = None      # (nc, names) cache — compile once per process


def _build_program():
    import concourse.bacc as bacc
    import concourse.bass as bass
    from concourse import mybir, tile
    from concourse.bass import ts

    f32 = mybir.dt.float32
    Identity = mybir.ActivationFunctionType.Identity

    nc = bacc.Bacc(
        "TRN2",
        target_bir_lowering=False,
        debug=False,
        enable_asserts=False,
        num_devices=8,
    )

    xr = nc.dram_tensor("xr", [CIN, NT], f32, kind="ExternalInput")
    att = nc.dram_tensor("att", [NI, N, T], f32, kind="ExternalInput")
    w1d = nc.dram_tensor("w1d", [128, HID], f32, kind="ExternalInput")
    b1d = nc.dram_tensor("b1d", [128, 1], f32, kind="ExternalInput")
    wv = nc.dram_tensor("wv", [HID, COUT], f32, kind="ExternalInput")
    whd = nc.dram_tensor("whd", [128, COUT], f32, kind="ExternalInput")
    b2c = nc.dram_tensor("b2c", [COUT, 1], f32, kind="ExternalInput")
    out = nc.dram_tensor("out", [COUT, IT], f32, kind="ExternalOutput")

    with tile.TileContext(nc) as tc:
        with (
            tc.tile_pool(name="const", bufs=1) as cpool,
            tc.tile_pool(name="data", bufs=1) as dpool,
            tc.tile_pool(name="pswork", bufs=2, space="PSUM") as pswork,
            tc.tile_pool(name="psbig", bufs=1, space="PSUM") as psbig,
            tc.tile_pool(name="psae", bufs=1, space="PSUM") as psae,
        ):
            w1t = cpool.tile([128, HID], f32)
            nc.sync.dma_start(w1t[:], w1d[:])
            b1t = cpool.tile([128, 1], f32)
            nc.sync.dma_start(b1t[:], b1d[:])
            wvt = cpool.tile([HID, COUT], f32)
            nc.sync.dma_start(wvt[:], wv[:])
            wht = cpool.tile([128, COUT], f32)
            nc.sync.dma_start(wht[:], whd[:])
            b2t = cpool.tile([COUT, 1], f32)
            nc.sync.dma_start(b2t[:], b2c[:])
            ones = cpool.tile([128, COUT], f32)
            nc.vector.memset(ones[:], 1.0)

            # x[b] as [128=(half,cin), 1536]: halves are the two 64-node
            # groups, so each MM half contracts cin on its own partitions.
            xT2 = dpool.tile([128, F2], f32)
            nc.sync.dma_start(xT2[:], xr[:].rearrange("c (h f) -> h c f", h=2))

            # attention transposed load: partition=j, free=(i,t)
            attT = dpool.tile([128, IT], f32)
            nc.sync.dma_start(attT[:], att[:].rearrange("i j t -> j i t"))

            hT2 = dpool.tile([128, F2], f32)
            mh = dpool.tile([128, F2], f32)
            mvb2 = dpool.tile([COUT, IT], f32)
            term1 = dpool.tile([COUT, IT], f32)
            updT = dpool.tile([COUT, IT], f32)

            # ---- h = x^T @ W1 + b1, as hT2[(half,hid), (n,t)] ----
            for f in range(3):
                ps_h = pswork.tile([128, 512], f32, tag="w")
                nc.tensor.matmul(
                    ps_h[0:64, :], w1t[0:64, :], xT2[0:64, ts(f, 512)],
                    start=True, stop=True,
                )
                nc.tensor.matmul(
                    ps_h[64:128, :], w1t[64:128, :], xT2[64:128, ts(f, 512)],
                    start=True, stop=True, tile_position=(64, 64),
                )
                nc.scalar.activation(
                    hT2[:, ts(f, 512)], ps_h[:], Identity, bias=b1t[:, 0:1]
                )

            # ---- mv+b2 for the dest chunk (rotated nodes 0..31 = cols 0..767
            # of half 0) ----
            for off, sz in ((0, 512), (512, 256)):
                ps_v = pswork.tile([128, 512], f32, tag="w")
                nc.tensor.matmul(
                    ps_v[0:64, 0:sz], wvt[:], hT2[0:64, off:off + sz],
                    start=True, stop=True,
                )
                nc.scalar.activation(
                    mvb2[:, off:off + sz], ps_v[0:64, 0:sz], Identity,
                    bias=b2t[:, 0:1],
                )

            # ---- asum broadcast over c: ones[j,c].T @ attT[j,(i,t)] ----
            ps_a = psae.tile([COUT, IT], f32, tag="ae")
            nc.tensor.matmul(ps_a[:, 0:512], ones[:], attT[:, 0:512],
                             start=True, stop=True)
            nc.tensor.matmul(ps_a[:, 512:768], ones[:], attT[:, 512:768],
                             start=True, stop=True)
            nc.vector.tensor_mul(term1[:], mvb2[:], ps_a[:])

            # ---- mh in j-partition layout: per t, two concurrent quadrant
            # MMs  lhsT=hT2[half, t-slice] (M=j_local), rhs=W2h ----
            hT2v = hT2[:].rearrange("p (n t) -> p t n", t=T)   # [128, 24, 64]
            ps_m = psbig.tile([128, F2], f32)
            for t in range(T):
                sl = slice(t * 64, t * 64 + 64)
                nc.tensor.matmul(
                    ps_m[0:64, sl], hT2v[0:64, t, :], wht[0:64, :],
                    start=True, stop=True,
                )
                nc.tensor.matmul(
                    ps_m[64:128, sl], hT2v[64:128, t, :], wht[64:128, :],
                    start=True, stop=True, tile_position=(64, 64),
                )
                if t % 8 == 7:
                    bank = t // 8
                    nc.vector.tensor_copy(mh[:, ts(bank, 512)],
                                          ps_m[:, ts(bank, 512)])

            # ---- einsum: per t, psum[c, i] += mh_t[j,c] x attT_t[j,i] ----
            attTv = attT[:].rearrange("j (i t) -> j t i", t=T)  # [128, 24, 32]
            ps_e = psae.tile([COUT, IT], f32, tag="ae")
            for t in range(T):
                nc.tensor.matmul(
                    ps_e[:, t * NI:(t + 1) * NI], mh[:, ts(t, 64)],
                    attTv[:, t, :], start=True, stop=True,
                )

            # ---- upd[c,(i,t)] = ps_e[c,(t,i)] + term1[c,(i,t)]; store ----
            ps_e_v = ps_e[:].rearrange("c (t i) -> c i t", i=NI)
            t1v = term1[:].rearrange("c (i t) -> c i t", t=T)
            updTv = updT[:].rearrange("c (i t) -> c i t", t=T)
            nc.vector.tensor_add(updTv, ps_e_v, t1v)
            nc.sync.dma_start(out[:], updT[:])

    nc.compile()
    return nc


def _get_program():
    global _PROGRAM
    if _PROGRAM is None:
        _PROGRAM = _build_program()
    return _PROGRAM


def _make_in_maps(x, attention, W1, b1, W2, b2):
    x = np.ascontiguousarray(x, dtype=np.float32)
    attention = np.ascontiguousarray(attention, dtype=np.float32)
    W1 = np.asarray(W1, dtype=np.float32)
    b1 = np.asarray(b1, dtype=np.float32)
    W2 = np.asarray(W2, dtype=np.float32)
    b2 = np.asarray(b2, dtype=np.float32)

    w1d = np.ascontiguousarray(np.concatenate([W1, W1], axis=0))      # [128,64]
    b1d = np.ascontiguousarray(np.concatenate([b1, b1])[:, None])     # [128,1]
    wv = np.ascontiguousarray(W2[:HID])                               # [64,64]
    whd = np.ascontiguousarray(np.concatenate([W2[HID:], W2[HID:]]))  # [128,64]
    b2c = np.ascontiguousarray(b2[:, None])                           # [64,1]

    in_maps = []
    for k in range(8):
        b, q = k // 4, k % 4
        i0 = NI * q
        # rotate node axis so this core's dest chunk sits at positions 0..31;
        # j axis of the attention slice rotated identically to stay aligned.
        xb = np.ascontiguousarray(
            np.roll(x[b], -i0, axis=1).reshape(CIN, NT))
        att_c = np.ascontiguousarray(
            np.roll(attention[b, 0, i0:i0 + NI], -i0, axis=1))
        in_maps.append({
            "xr": xb, "att": att_c, "w1d": w1d, "b1d": b1d,
            "wv": wv, "whd": whd, "b2c": b2c,
        })
    return in_maps


def run(inputs: dict, trace: bool = False):
    """Compile (cached), shard, run on 8 cores; returns (full_out, results)."""
    from concourse import bass_utils

    nc = _get_program()
    in_maps = _make_in_maps(**inputs)
    res = bass_utils.run_bass_kernel_spmd(
        nc, in_maps, core_ids=list(range(8)), trace=trace,
    )
    full = np.empty((B, COUT, N, T), dtype=np.float32)
    for k in range(8):
        b, q = k // 4, k % 4
        i0 = NI * q
        full[b, :, i0:i0 + NI, :] = res.results[k]["out"].reshape(COUT, NI, T)
    return full, res


def kernel(**inputs) -> np.ndarray:
    full, _ = run(inputs, trace=False)
    return full


# revision 5
# speedup vs baseline: 2.4838x; 2.4838x over previous
"""Trainium2 Bass kernel for nn_MessageAttentionPassing.

Math (reference):
    xp  = x.transpose(0,2,3,1)            # [B, N, T, CIN]
    h   = xp @ W1 + b1                    # [B, N, T, HID]
    mv  = h @ W2[:HID]                    # dest part
    mh  = h @ W2[HID:]                    # src part
    a    = attention[:, 0]                # [B, N(i), N(j), T]
    asum = a.sum(axis=2)                  # [B, N, T]
    upd = asum[...,None]*(mv+b2) + einsum('bijt,bjtc->bitc', a, mh)
    out = upd.transpose(0,3,1,2)          # [B, COUT, N, T]

Sharding: 8 cores = (batch b in {0,1}) x (dest-node quarter q in {0..3}).
Each core loads the full x[b] (to build h/mh over all source nodes j) and
its 32-row attention slice.  Inputs are node-rotated by i0 = 32*q on the
host so every core runs the IDENTICAL program with its own dest chunk at
rotated positions 0..31 (run_bass_kernel_spmd requires one shared BIR
program across cores).

On-chip layouts (per core, rotated node axis n, free dim (n,t) n-major):
    xT2  [128, 1536]  partition=(half,cin)  free=local (n,t), halves n<64 / n>=64
    hT2  [128, 1536]  partition=(half,hid)  same free split
    mh   [128, 1536]  partition=j (all 128) free=(t,c) t-major
    attT [128,  768]  partition=j           free=(i,t) i-major (DMA-transposed load)
    per-t einsum: psum[c, i] = mh_t[j,c].T-weights @ attT_t[j,i]
"""

import os
import sys
import numpy as np

if "/opt/trn_rl_repo" not in sys.path:
    sys.path.insert(0, "/opt/trn_rl_repo")

B, CIN, N, T, COUT, HID = 2, 64, 128, 24, 64, 64
NI = N // 4          # dest-node chunk per core: 32
NT = N * T           # 3072
F2 = NT // 2         # 1536
IT = NI * T          # 768

_PROGRAM = None      # compiled program cache — compile once per process


def _build_program(reps: int = 1):
    import concourse.bacc as bacc
    import concourse.bass as bass
    from concourse import mybir, tile
    from concourse.bass import ts

    f32 = mybir.dt.float32
    Identity = mybir.ActivationFunctionType.Identity

    nc = bacc.Bacc(
        "TRN2",
        target_bir_lowering=False,
        debug=False,
        enable_asserts=False,
        num_devices=8,
    )

    xr = nc.dram_tensor("xr", [CIN, NT], f32, kind="ExternalInput")
    att = nc.dram_tensor("att", [NI, N, T], f32, kind="ExternalInput")
    w1d = nc.dram_tensor("w1d", [128, HID], f32, kind="ExternalInput")
    b1d = nc.dram_tensor("b1d", [128, 1], f32, kind="ExternalInput")
    wv = nc.dram_tensor("wv", [HID, COUT], f32, kind="ExternalInput")
    whd = nc.dram_tensor("whd", [128, COUT], f32, kind="ExternalInput")
    b2c = nc.dram_tensor("b2c", [COUT, 1], f32, kind="ExternalInput")
    out = nc.dram_tensor("out", [COUT, IT], f32, kind="ExternalOutput")

    with tile.TileContext(nc) as tc:
        with (
            tc.tile_pool(name="const", bufs=1) as cpool,
            tc.tile_pool(name="data", bufs=2) as dpool,
            tc.tile_pool(name="pswork", bufs=2, space="PSUM") as pswork,
            tc.tile_pool(name="psbig", bufs=1, space="PSUM") as psbig,
            tc.tile_pool(name="psae", bufs=1, space="PSUM") as psae,
        ):
            w1t = cpool.tile([128, HID], f32)
            nc.sync.dma_start(w1t[:], w1d[:])
            b1t = cpool.tile([128, 1], f32)
            nc.sync.dma_start(b1t[:], b1d[:])
            wvt = cpool.tile([HID, COUT], f32)
            nc.sync.dma_start(wvt[:], wv[:])
            wht = cpool.tile([128, COUT], f32)
            nc.sync.dma_start(wht[:], whd[:])
            b2t = cpool.tile([COUT, 1], f32)
            nc.sync.dma_start(b2t[:], b2c[:])
            ones = cpool.tile([128, COUT], f32)
            nc.vector.memset(ones[:], 1.0)

            for _rep in range(reps):
                _rep_body(nc, tc, dpool, pswork, psbig, psae,
                          xr, att, out, w1t, b1t, wvt, wht, b2t, ones,
                          f32, Identity, ts)

    nc.compile()
    return nc


def _rep_body(nc, tc, dpool, pswork, psbig, psae,
              xr, att, out, w1t, b1t, wvt, wht, b2t, ones,
              f32, Identity, ts):
    T_ = T
    if True:
        if True:
            # x[b] as [128=(half,cin), 1536]: halves are the two 64-node
            # groups, so each MM half contracts cin on its own partitions.
            xT2 = dpool.tile([128, F2], f32, tag="xT2")
            nc.sync.dma_start(xT2[:], xr[:].rearrange("c (h f) -> h c f", h=2))

            # attention transposed load: partition=j, free=(i,t)
            attT = dpool.tile([128, IT], f32, tag="attT")
            nc.sync.dma_start(attT[:], att[:].rearrange("i j t -> j i t"))

            hT2 = dpool.tile([128, F2], f32, tag="hT2")
            mh = dpool.tile([128, F2], f32, tag="mh")
            mvb2 = dpool.tile([COUT, IT], f32, tag="mvb2")
            term1 = dpool.tile([COUT, IT], f32, tag="term1")
            updT = dpool.tile([COUT, IT], f32, tag="updT")

            # ---- h = x^T @ W1 + b1, as hT2[(half,hid), (n,t)] ----
            for f in range(3):
                ps_h = pswork.tile([128, 512], f32, tag="w")
                nc.tensor.matmul(
                    ps_h[0:64, :], w1t[0:64, :], xT2[0:64, ts(f, 512)],
                    start=True, stop=True,
                )
                nc.tensor.matmul(
                    ps_h[64:128, :], w1t[64:128, :], xT2[64:128, ts(f, 512)],
                    start=True, stop=True, tile_position=(64, 64),
                )
                nc.scalar.activation(
                    hT2[:, ts(f, 512)], ps_h[:], Identity, bias=b1t[:, 0:1]
                )

            # ---- mv+b2 for the dest chunk (rotated nodes 0..31 = cols 0..767
            # of half 0) ----
            for off, sz in ((0, 512), (512, 256)):
                ps_v = pswork.tile([128, 512], f32, tag="w")
                nc.tensor.matmul(
                    ps_v[0:64, 0:sz], wvt[:], hT2[0:64, off:off + sz],
                    start=True, stop=True,
                )
                nc.scalar.activation(
                    mvb2[:, off:off + sz], ps_v[0:64, 0:sz], Identity,
                    bias=b2t[:, 0:1],
                )

            # ---- asum broadcast over c: ones[j,c].T @ attT[j,(i,t)] ----
            ps_a = psae.tile([COUT, IT], f32, tag="ae")
            nc.tensor.matmul(ps_a[:, 0:512], ones[:], attT[:, 0:512],
                             start=True, stop=True)
            nc.tensor.matmul(ps_a[:, 512:768], ones[:], attT[:, 512:768],
                             start=True, stop=True)
            nc.vector.tensor_mul(term1[:], mvb2[:], ps_a[:])

            # ---- mh in j-partition layout: per t, two concurrent quadrant
            # MMs  lhsT=hT2[half, t-slice] (M=j_local), rhs=W2h ----
            hT2v = hT2[:].rearrange("p (n t) -> p t n", t=T_)  # [128, 24, 64]
            ps_m = psbig.tile([128, F2], f32, tag="ps_m")
            for t in range(T_):
                sl = slice(t * 64, t * 64 + 64)
                nc.tensor.matmul(
                    ps_m[0:64, sl], hT2v[0:64, t, :], wht[0:64, :],
                    start=True, stop=True,
                )
                nc.tensor.matmul(
                    ps_m[64:128, sl], hT2v[64:128, t, :], wht[64:128, :],
                    start=True, stop=True, tile_position=(64, 64),
                )
                if t % 8 == 7:
                    bank = t // 8
                    nc.vector.tensor_copy(mh[:, ts(bank, 512)],
                                          ps_m[:, ts(bank, 512)])

            # ---- einsum: per t, psum[c, i] += mh_t[j,c] x attT_t[j,i] ----
            attTv = attT[:].rearrange("j (i t) -> j t i", t=T)  # [128, 24, 32]
            ps_e = psae.tile([COUT, IT], f32, tag="ae")
            for t in range(T):
                nc.tensor.matmul(
                    ps_e[:, t * NI:(t + 1) * NI], mh[:, ts(t, 64)],
                    attTv[:, t, :], start=True, stop=True,
                )

            # ---- upd[c,(i,t)] = ps_e[c,(t,i)] + term1[c,(i,t)]; store ----
            ps_e_v = ps_e[:].rearrange("c (t i) -> c i t", i=NI)
            t1v = term1[:].rearrange("c (i t) -> c i t", t=T)
            updTv = updT[:].rearrange("c (i t) -> c i t", t=T)
            nc.vector.tensor_add(updTv, ps_e_v, t1v)
            nc.sync.dma_start(out[:], updT[:])


def _get_program():
    global _PROGRAM
    if _PROGRAM is None:
        _PROGRAM = _build_program()
    return _PROGRAM


def _make_in_maps(x, attention, W1, b1, W2, b2):
    x = np.ascontiguousarray(x, dtype=np.float32)
    attention = np.ascontiguousarray(attention, dtype=np.float32)
    W1 = np.asarray(W1, dtype=np.float32)
    b1 = np.asarray(b1, dtype=np.float32)
    W2 = np.asarray(W2, dtype=np.float32)
    b2 = np.asarray(b2, dtype=np.float32)

    w1d = np.ascontiguousarray(np.concatenate([W1, W1], axis=0))      # [128,64]
    b1d = np.ascontiguousarray(np.concatenate([b1, b1])[:, None])     # [128,1]
    wv = np.ascontiguousarray(W2[:HID])                               # [64,64]
    whd = np.ascontiguousarray(np.concatenate([W2[HID:], W2[HID:]]))  # [128,64]
    b2c = np.ascontiguousarray(b2[:, None])                           # [64,1]

    in_maps = []
    for k in range(8):
        b, q = k // 4, k % 4
        i0 = NI * q
        # rotate node axis so this core's dest chunk sits at positions 0..31;
        # j axis of the attention slice rotated identically to stay aligned.
        xb = np.ascontiguousarray(
            np.roll(x[b], -i0, axis=1).reshape(CIN, NT))
        att_c = np.ascontiguousarray(
            np.roll(attention[b, 0, i0:i0 + NI], -i0, axis=1))
        in_maps.append({
            "xr": xb, "att": att_c, "w1d": w1d, "b1d": b1d,
            "wv": wv, "whd": whd, "b2c": b2c,
        })
    return in_maps


def run(inputs: dict, trace: bool = False):
    """Compile (cached), shard, run on 8 cores; returns (full_out, results)."""
    from concourse import bass_utils

    nc = _get_program()
    in_maps = _make_in_maps(**inputs)
    res = bass_utils.run_bass_kernel_spmd(
        nc, in_maps, core_ids=list(range(8)), trace=trace,
    )
    full = np.empty((B, COUT, N, T), dtype=np.float32)
    for k in range(8):
        b, q = k // 4, k % 4
        i0 = NI * q
        full[b, :, i0:i0 + NI, :] = res.results[k]["out"].reshape(COUT, NI, T)
    return full, res


def kernel(**inputs) -> np.ndarray:
    full, _ = run(inputs, trace=False)
    return full
